# revision 51
# baseline (speedup 1.0000x reference)
"""GGNN message-passing kernel on 8 TRN2 NeuronCores.

Sharding: 50000 nodes padded to 51200, 6400 per core. Per GGC step:
AllGather h (bf16, 2 half-tables of 25600 rows so int16 gather indices
fit; each AG fires as soon as its half of agin is stored, overlapping
the previous step), then one dma_gather per (super-tile, half) cell
whose trailing -1 pad indices are skipped at runtime via a per-core
count register (desc-gen cost = real edges only), segment-sum via
256-slot one-hot matmuls on PE (one-hot shipped as fp8 e5m2 - 0/1
exact - halving its DMA), fused GRU gate matmuls (r/z gates accumulate
gi+gh in one PSUM group; V_s = W_ggc[s] @ W_ih.T folded on host), GRU
elementwise on ACT/DVE, PE-transpose h back to row-major for the next
AllGather. Gather-pool slots are memset once: reg-skipped tails stay
stale and 0.0 x NaN from uninitialized SBUF would poison PSUM.
"""

import os

import numpy as np
import ml_dtypes

import concourse.bass as bass
import concourse.mybir as mybir
import concourse.tile as tile
from concourse import bacc
from concourse import bass_utils
from concourse.tile import add_dep_helper

P = 128
NCORES = 8
N_REAL = 50000
N_PAD = 51200
SHARD = N_PAD // NCORES          # 6400
HALF_LOCAL = SHARD // 2          # 3200 rows/core contributed to each table
TBL = HALF_LOCAL * NCORES        # 25600 rows per gather table (int16-safe)
NT = SHARD // P                  # 50 dst tiles / core
NST = NT // 2                    # 25 super-tiles of 256 nodes
ANNOT = 200
HID = 256
G3 = 768
STEPS = 8
SWDGE_SCRATCH = 16384            # descriptor-ring carveout (1024 descs)
SWDGE_QUEUES = 4

f32 = mybir.dt.float32
bf16 = mybir.dt.bfloat16
i16 = mybir.dt.int16

AF = mybir.ActivationFunctionType
ALU = mybir.AluOpType


# ----------------------------------------------------------------- host prep

def _prep_edges(edge_index):
    """Per-core gather indices / one-hot slots with SPMD-uniform structure.

    Returns (caps, percore) where caps[t][h] = chunk count for dst-tile t,
    source-half h (same for all cores), and percore[c] = dict with wrapped
    int16 index buffers and f32 slot columns for both halves.
    """
    src = np.asarray(edge_index[0], dtype=np.int64)
    dst = np.asarray(edge_index[1], dtype=np.int64)

    core_of = dst // SHARD
    dst_local = dst % SHARD
    t_of = dst_local // 256          # super-tile (256 dst slots) of dst
    slot_of = dst_local % 256
    src_half = (src % SHARD) // HALF_LOCAL
    src_row = (src // SHARD) * HALF_LOCAL + (src % HALF_LOCAL)

    # bucket edges: cells[c][t][h] -> (rows, slots)
    cells = [[[None, None] for _ in range(NST)] for _ in range(NCORES)]
    order = np.lexsort((slot_of, src_half, t_of, core_of))
    c_s, t_s, h_s = core_of[order], t_of[order], src_half[order]
    r_s, sl_s = src_row[order], slot_of[order]
    key = ((c_s * NST + t_s) * 2 + h_s)
    bounds = np.searchsorted(key, np.arange(NCORES * NST * 2 + 1))
    for c in range(NCORES):
        for t in range(NST):
            for h in range(2):
                k = (c * NST + t) * 2 + h
                a, b = bounds[k], bounds[k + 1]
                cells[c][t][h] = (r_s[a:b], sl_s[a:b])

    caps = np.zeros((NST, 2), dtype=np.int64)
    for t in range(NST):
        for h in range(2):
            mx = max(len(cells[c][t][h][0]) for c in range(NCORES))
            caps[t, h] = (mx + P - 1) // P
        if caps[t, 0] == 0:
            caps[t, 0] = 1  # guarantee >=1 chunk per tile (PSUM init)

    percore = []
    for c in range(NCORES):
        d = {}
        cnt_th = np.zeros((NST, 2), dtype=np.uint32)
        for h in range(2):
            rows_l, slots_l = [], []
            for t in range(NST):
                r, sl = cells[c][t][h]
                n_pad = caps[t, h] * P
                if n_pad == 0:
                    continue
                # real rows first, trailing -1 pads are SKIPPED by the DMA
                # (num_idxs_reg = per-core valid count, loaded at runtime)
                rp = np.full(n_pad, -1, dtype=np.int64)
                sp = np.full(n_pad, -1.0, dtype=np.float32)
                rp[: len(r)] = r
                sp[: len(sl)] = sl
                if len(r) == 0:
                    rp[0] = 0  # keep >=1 valid idx per gather
                    cnt_th[t, h] = 1
                else:
                    cnt_th[t, h] = len(r)
                rows_l.append(rp)
                slots_l.append(sp)
            rows = np.concatenate(rows_l) if rows_l else np.zeros(0, np.int64)
            slots = np.concatenate(slots_l)
            nch = len(rows) // P
            # wrapped int16 index buffer, replicated to 8 groups of 16 parts
            wb = np.zeros((16, len(rows) // 16), dtype=np.int16)
            ar = np.arange(len(rows))
            wb[ar % 16, ar // 16] = rows.astype(np.int16)
            d[f"idx{h}"] = np.tile(wb, (8, 1))
            # one-hot blocks [p, chunk, c]: block[p, ci, c] = (slot == c),
            # c over the 256 slots of the super-tile; fp8 (0/1 exact)
            sl2 = slots.reshape(nch, P)  # [ci, p]
            oh = (
                sl2[:, :, None] == np.arange(256, dtype=np.float32)[None, None, :]
            ).astype(ml_dtypes.float8_e5m2)
            d[f"oh{h}"] = oh.transpose(1, 0, 2).copy()  # [p, ci, c]
        # cell order on device: t-major, h inner (skipping cap==0 cells)
        cnts = [
            cnt_th[t, h]
            for t in range(NST)
            for h in range(2)
            if caps[t, h] > 0
        ]
        d["cnt"] = np.asarray(cnts, dtype=np.uint32).reshape(1, -1)
        percore.append(d)
    return caps, percore


def _cells(caps, h):
    """Per-half gather cells: (chunk offset, cap chunks) per super-tile."""
    out = []
    c0 = 0
    for t in range(NST):
        k = int(caps[t, h])
        if k > 0:
            out.append((c0, k))
        c0 += k
    return out


# ------------------------------------------------------------- device build

def _build_nc(caps, C0, C1, b_d=0.0, dense_dtype=bf16):
    nc = bacc.Bacc(
        "TRN2",
        target_bir_lowering=False,
        num_devices=NCORES,
        dynamic_dma_scratch_size=SWDGE_SCRATCH,
        num_swdge_queues=SWDGE_QUEUES,
    )
    cdt = dense_dtype
    n_steps = int(os.environ.get("KERNEL_STEPS", STEPS))
    dbg_h = os.environ.get("KERNEL_DEBUG_H") == "1"

    # ---- I/O ----
    xT_in = nc.dram_tensor("xT", [P, 2, SHARD], bf16, kind="ExternalInput")
    wred_in = nc.dram_tensor("w_red", [P, 2, HID], bf16, kind="ExternalInput")
    v_in = nc.dram_tensor("v", [STEPS, P, 2, G3], cdt, kind="ExternalInput")
    whh_in = nc.dram_tensor("whh", [P, 2, G3], cdt, kind="ExternalInput")
    brz_in = nc.dram_tensor("b_rz", [P, 4], f32, kind="ExternalInput")
    bin_in = nc.dram_tensor("b_in", [P, 2], f32, kind="ExternalInput")
    bhn_in = nc.dram_tensor("b_hn", [P, 2], f32, kind="ExternalInput")
    bred_in = nc.dram_tensor("b_red", [P, 2], f32, kind="ExternalInput")
    wd_in = nc.dram_tensor("w_d", [P, 2, 1], bf16, kind="ExternalInput")
    idx_in = [
        nc.dram_tensor("idx0", [P, C0 * 8], i16, kind="ExternalInput"),
        nc.dram_tensor("idx1", [P, C1 * 8], i16, kind="ExternalInput"),
    ]
    f8 = mybir.dt.float8e5
    oh_in = [
        nc.dram_tensor("oh0", [P, C0, 256], f8, kind="ExternalInput"),
        nc.dram_tensor("oh1", [P, C1, 256], f8, kind="ExternalInput"),
    ]
    ncell = len(_cells(caps, 0)) + len(_cells(caps, 1))
    cnt_in = nc.dram_tensor("cnt", [1, ncell], mybir.dt.uint32, kind="ExternalInput")
    out_t = nc.dram_tensor("out", [2, SHARD], f32, kind="ExternalOutput")
    h_out = (
        nc.dram_tensor("h_out", [P, 2, SHARD], bf16, kind="ExternalOutput")
        if dbg_h
        else None
    )
    agg_out = (
        nc.dram_tensor("agg_out", [NST, P, 2, 256], bf16, kind="ExternalOutput")
        if dbg_h
        else None
    )

    CH = [C0, C1]
    CAPMAX = int(caps.max())
    assert CAPMAX <= 8, f"gather cell over 1024 rows: {CAPMAX}"
    # chunk index -> (super-tile, start/stop flags); order: t asc, h0 then h1
    chunk_tile = [[], []]
    chunk_flags = [[], []]  # (is_first_of_tile, is_last_of_tile)
    for h in range(2):
        for t in range(NST):
            k = int(caps[t, h])
            for j in range(k):
                first = (h == 0 and j == 0)
                last = (j == k - 1) and (h == 1 or caps[t, 1] == 0)
                chunk_tile[h].append(t)
                chunk_flags[h].append((first, last))

    with tile.TileContext(nc) as tc:
        import contextlib

        est = contextlib.ExitStack()
        with est:
            sbr = est.enter_context(tc.tile_pool(name="resident", bufs=1))
            dram = est.enter_context(tc.tile_pool(name="dram", bufs=1, space="DRAM"))
            main = contextlib.ExitStack()
            sbw = main.enter_context(tc.tile_pool(name="work", bufs=3))
            sbg = main.enter_context(tc.tile_pool(name="gat", bufs=3))
            sboh = main.enter_context(tc.tile_pool(name="oh", bufs=6))
            ps_agg = main.enter_context(
                tc.tile_pool(name="ps_agg", bufs=2, space="PSUM")
            )
            ps_gru = main.enter_context(
                tc.tile_pool(name="ps_gru", bufs=1, space="PSUM")
            )
            ps_tr = main.enter_context(
                tc.tile_pool(name="ps_tr", bufs=2, space="PSUM")
            )

            # ---- residents ----
            hT = sbr.tile([P, 2, SHARD], bf16, name="hT")
            idx_t = [
                sbr.tile([P, C0 * 8], i16, name="idx_t0"),
                sbr.tile([P, C1 * 8], i16, name="idx_t1"),
            ]
            whh = sbr.tile([P, 2, G3], cdt, name="whh")
            brz = sbr.tile([P, 4], f32, name="brz")
            bin_ = sbr.tile([P, 2], f32, name="bin")
            bhn = sbr.tile([P, 2], f32, name="bhn")
            bred = sbr.tile([P, 2], f32, name="bred")
            wd = sbr.tile([P, 2, 1], bf16, name="wd")
            iota = sbr.tile([P, P], f32, name="iota")
            ident = sbr.tile([P, P], bf16, name="ident")
            iota_c = sbr.tile([P, 1], f32, name="iota_c")
            cnt_t = sbr.tile([1, ncell], mybir.dt.uint32, name="cnt_t")
            gcnt_reg = [
                nc.gpsimd.alloc_register(f"gcnt{h}") for h in range(2)
            ]

            for h in range(2):
                nc.sync.dma_start(idx_t[h][:], idx_in[h][:])
            nc.sync.dma_start(cnt_t[:], cnt_in[:])
            nc.sync.dma_start(whh[:], whh_in[:])
            nc.sync.dma_start(brz[:], brz_in[:])
            nc.sync.dma_start(bin_[:], bin_in[:])
            nc.sync.dma_start(bhn[:], bhn_in[:])
            nc.sync.dma_start(bred[:], bred_in[:])
            nc.sync.dma_start(wd[:], wd_in[:])
            nc.gpsimd.iota(
                iota[:], pattern=[[1, P]], base=0, channel_multiplier=0,
                allow_small_or_imprecise_dtypes=True,
            )
            nc.gpsimd.iota(
                iota_c[:], pattern=[[1, 1]], base=0, channel_multiplier=1,
                allow_small_or_imprecise_dtypes=True,
            )
            nc.vector.tensor_scalar(
                ident[:], iota[:], iota_c[:, 0:1], None, ALU.is_equal
            )

            # collective bounce buffers (two half-table AllGathers per step)
            agin = [
                dram.tile([HALF_LOCAL, HID], bf16, name=f"agin{h}")
                for h in range(2)
            ]
            agout = [
                [
                    dram.tile(
                        [TBL, HID], bf16, name=f"agout{s}_{h}",
                        addr_space="Shared",
                    )
                    for h in range(2)
                ]
                for s in range(STEPS)
            ]

            v_pool = main.enter_context(tc.tile_pool(name="vpool", bufs=2))

            def start_ag(sdx, h):
                nc.gpsimd.collective_compute(
                    "AllGather",
                    ALU.bypass,
                    replica_groups=[list(range(NCORES))],
                    ins=[agin[h].opt()],
                    outs=[agout[sdx][h].opt()],
                )

            def transpose_store(st, last_step, ag_sdx):
                """PE-transpose hT super-tile st to row-major, DMA to agin.

                Fires AG(ag_sdx, 0) as soon as agin half 0 is complete
                (after st==12) so its wire time overlaps the rest of this
                step, and AG(ag_sdx, 1) after the last super-tile.
                """
                if last_step:
                    return
                for nb in (2 * st, 2 * st + 1):
                    trp = ps_tr.tile([P, HID], bf16, name=f"trp{nb}", tag="trp")
                    for k in range(2):
                        nc.tensor.transpose(
                            trp[:, k * P : (k + 1) * P],
                            hT[:, k, nb * P : (nb + 1) * P],
                            ident[:],
                        )
                    stg = sbw.tile([P, HID], bf16, name=f"stg{nb}", tag="stg")
                    nc.scalar.copy(stg[:], trp[:])
                    h = nb // (NT // 2)
                    r0 = (nb - h * (NT // 2)) * P
                    nc.sync.dma_start(agin[h][r0 : r0 + P, :], stg[:])
                if st == 12:
                    start_ag(ag_sdx, 0)
                elif st == NST - 1:
                    start_ag(ag_sdx, 1)

            # Zero the gather-pool slots once: reg-skipped tail regions are
            # never DMA'd, and uninitialized SBUF can hold NaN bit patterns
            # that poison the PSUM bracket even through a 0.0 one-hot column.
            for hh in range(2):
                for b in range(4):
                    warm = sbg.tile(
                        [P, CAPMAX, HID], bf16,
                        name=f"warm{hh}_{b}", tag=f"gat{hh}", bufs=4,
                    )
                    nc.gpsimd.memset(warm[:], 0.0)

            # ---- init: h0 = W_red.T @ xT + b_red ----
            with tc.tile_pool(name="xbuf", bufs=2) as xpool:
                wred = sbr.tile([P, 2, HID], bf16, name="wred")
                nc.sync.dma_start(wred[:], wred_in[:])
                for st in range(NST):
                    sl = slice(st * 2 * P, (st + 1) * 2 * P)
                    xt = xpool.tile([P, 2, 2 * P], bf16, name=f"x{st}", tag="x")
                    nc.sync.dma_start(xt[:], xT_in[:, :, sl])
                    hp = ps_agg.tile([P, 512], f32, name=f"h0p{st}", tag="agg")
                    si = None
                    for m in range(2):
                        for kc in range(2):
                            i = m * 2 + kc
                            mm = nc.tensor.matmul(
                                hp[:, m * 256 : (m + 1) * 256],
                                wred[:, kc, m * P : (m + 1) * P],
                                xt[:, kc, :],
                                start=(i == 0),
                                stop=(i == 3),
                            )
                            if i == 0:
                                si = mm.ins
                            else:
                                add_dep_helper(
                                    mm.ins, si, reason="psum bracket order"
                                )
                    for m in range(2):
                        nc.scalar.activation(
                            hT[:, m, sl],
                            hp[:, m * 256 : (m + 1) * 256],
                            AF.Identity,
                            bias=bred[:, m : m + 1],
                        )
                    transpose_store(st, False, 0)

            # ---- GGC steps ----
            for s in range(n_steps):
                vt = v_pool.tile([P, 2, G3], cdt, name=f"v{s}", tag="v")
                nc.sync.dma_start(vt[:], v_in[s])

                gat_tiles = [[], []]  # per half: (c0, n, tile, oh_tile)
                cell_i = 0
                cell_maps = [dict(), dict()]
                for h in range(2):
                    c0 = 0
                    for t in range(NST):
                        k = int(caps[t, h])
                        if k > 0:
                            cell_maps[h][t] = (c0, k)
                        c0 += k
                def emit_cells(t):
                    nonlocal cell_i
                    for h in range(2):
                        if t not in cell_maps[h]:
                            continue
                        (c0, n) = cell_maps[h][t]
                        # per-core valid-row count: DMA skips trailing -1 idxs
                        nc.gpsimd.reg_load(
                            gcnt_reg[h], cnt_t[0:1, cell_i : cell_i + 1]
                        )
                        g = sbg.tile(
                            [P, CAPMAX, HID], bf16,
                            name=f"g{s}_{h}_{c0}", tag=f"gat{h}", bufs=4,
                        )
                        nc.gpsimd.dma_gather(
                            g[:, :n, :],
                            agout[s][h][:],
                            idx_t[h][:, c0 * 8 : (c0 + n) * 8],
                            n * P,
                            gcnt_reg[h],
                            HID,
                            queue_num=cell_i % SWDGE_QUEUES,
                            single_packet=os.environ.get("KERNEL_SP", "1") == "1",
                        )
                        cell_i += 1
                        oht = sboh.tile(
                            [P, CAPMAX, 256], f8,
                            name=f"oh{s}_{h}_{c0}", tag=f"oh{h}", bufs=4,
                        )
                        nc.sync.dma_start(
                            oht[:, :n, :], oh_in[h][:, c0 : c0 + n, :]
                        )
                        gat_tiles[h].append((c0, n, g, oht))

                def chunk_src(h, ci):
                    for (c0, n, g, oht) in gat_tiles[h]:
                        if c0 <= ci < c0 + n:
                            return g, oht, ci - c0
                    raise AssertionError

                # per-tile chunk lists
                tile_chunks = [[] for _ in range(NST)]
                for h in range(2):
                    for ci, t in enumerate(chunk_tile[h]):
                        tile_chunks[t].append((h, ci) + chunk_flags[h][ci])

                for st in range(NST):
                    emit_cells(st)
                    sl = slice(st * 2 * P, (st + 1) * 2 * P)
                    # --- aggregation: ONE psum bracket per bank (zero-region
                    # semantics: start pends the whole 2KB bank) ---
                    agp = ps_agg.tile([P, 512], f32, name=f"agp{s}_{st}", tag="agg")
                    work = [
                        (h, ci) for (h, ci, _f, _l) in tile_chunks[st]
                    ]
                    n_mm = len(work) * 2
                    mm_i = 0
                    start_inst = None
                    for (h, ci) in work:
                        g, oht, gj = chunk_src(h, ci)
                        for k in range(2):
                            mm = nc.tensor.matmul(
                                agp[:, k * 256 : (k + 1) * 256],
                                g[:, gj, k * P : (k + 1) * P],
                                oht[:, gj, :],
                                start=(mm_i == 0),
                                stop=(mm_i == n_mm - 1),
                            )
                            if mm_i == 0:
                                start_inst = mm.ins
                            else:
                                add_dep_helper(
                                    mm.ins, start_inst, reason="psum bracket order"
                                )
                            mm_i += 1
                    aggb = sbw.tile([P, 2, 256], cdt, name=f"aggb{s}_{st}", tag="aggb", bufs=4)
                    for k in range(2):
                        nc.scalar.copy(aggb[:, k, :], agp[:, k * 256 : (k + 1) * 256])
                    if agg_out is not None and s == 0:
                        nc.sync.dma_start(agg_out[st], aggb[:])

                    # --- dense gate matmuls ---
                    rz = [
                        ps_gru.tile([P, 512], f32, name=f"rz{j}_{s}_{st}", tag=f"rz{j}")
                        for j in range(2)
                    ]
                    inn = ps_gru.tile([P, 512], f32, name=f"in_{s}_{st}", tag="inn")
                    hnn = ps_gru.tile([P, 512], f32, name=f"hn_{s}_{st}", tag="hnn")

                    def bracket(mms):
                        si = None
                        for i, f in enumerate(mms):
                            mm = f(i == 0, i == len(mms) - 1)
                            if i == 0:
                                si = mm.ins
                            else:
                                add_dep_helper(
                                    mm.ins, si, reason="psum bracket order"
                                )

                    def gi_mm(o, m, kc):
                        return lambda st_, sp_: nc.tensor.matmul(
                            o, vt[:, kc, m * P : (m + 1) * P],
                            aggb[:, kc, :], start=st_, stop=sp_,
                        )

                    def gh_mm(o, m, kc):
                        return lambda st_, sp_: nc.tensor.matmul(
                            o, whh[:, kc, m * P : (m + 1) * P],
                            hT[:, kc, sl], start=st_, stop=sp_,
                        )

                    for j in range(2):
                        mms = []
                        for mi in range(2):
                            m = 2 * j + mi
                            o = rz[j][:, mi * 256 : (mi + 1) * 256]
                            mms += [gi_mm(o, m, 0), gi_mm(o, m, 1),
                                    gh_mm(o, m, 0), gh_mm(o, m, 1)]
                        bracket(mms)
                    mms_i, mms_h = [], []
                    for m in (4, 5):
                        o = inn[:, (m - 4) * 256 : (m - 3) * 256]
                        o2 = hnn[:, (m - 4) * 256 : (m - 3) * 256]
                        mms_i += [gi_mm(o, m, 0), gi_mm(o, m, 1)]
                        mms_h += [gh_mm(o2, m, 0), gh_mm(o2, m, 1)]
                    bracket(mms_i)
                    bracket(mms_h)

                    # --- GRU elementwise ---
                    for k in range(2):
                        ksl = slice(k * 256, (k + 1) * 256)
                        r = sbw.tile([P, 256], f32, name=f"r{s}_{st}_{k}", tag="r")
                        nc.scalar.activation(
                            r[:], rz[0][:, ksl], AF.Sigmoid, bias=brz[:, k : k + 1]
                        )
                        z = sbw.tile([P, 256], f32, name=f"z{s}_{st}_{k}", tag="z")
                        nc.scalar.activation(
                            z[:], rz[1][:, ksl], AF.Sigmoid,
                            bias=brz[:, 2 + k : 3 + k],
                        )
                        hn = sbw.tile([P, 256], f32, name=f"hn{s}_{st}_{k}", tag="hn")
                        nc.scalar.activation(
                            hn[:], hnn[:, ksl], AF.Identity,
                            bias=bhn[:, k : k + 1],
                        )
                        t1 = sbw.tile([P, 256], f32, name=f"t1{s}_{st}_{k}", tag="t1")
                        nc.vector.tensor_mul(t1[:], r[:], hn[:])
                        t2 = sbw.tile([P, 256], f32, name=f"t2{s}_{st}_{k}", tag="t2")
                        nc.vector.tensor_add(t2[:], t1[:], inn[:, ksl])
                        n_ = sbw.tile([P, 256], f32, name=f"n{s}_{st}_{k}", tag="n")
                        nc.scalar.activation(
                            n_[:], t2[:], AF.Tanh, bias=bin_[:, k : k + 1]
                        )
                        d = sbw.tile([P, 256], f32, name=f"d{s}_{st}_{k}", tag="d")
                        nc.vector.tensor_sub(d[:], hT[:, k, sl], n_[:])
                        e = sbw.tile([P, 256], f32, name=f"e{s}_{st}_{k}", tag="e")
                        nc.vector.tensor_mul(e[:], z[:], d[:])
                        nc.vector.tensor_add(hT[:, k, sl], n_[:], e[:])

                    transpose_store(st, s == n_steps - 1, s + 1)

            main.close()
            if h_out is not None:
                nc.sync.dma_start(h_out[:], hT[:])

            # ---- head: out = log_softmax(relu(h) @ W_lin.T + b_lin) ----
            with (
                tc.tile_pool(name="head", bufs=3) as hp_pool,
                tc.tile_pool(name="head_ps", bufs=2, space="PSUM") as hps,
            ):
                d_sb = hp_pool.tile([1, SHARD], f32, name="d_sb", bufs=1)
                nch = (SHARD + 511) // 512
                for j in range(nch):
                    w = min(512, SHARD - j * 512)
                    jsl = slice(j * 512, j * 512 + w)
                    rl = hp_pool.tile([P, 2, 512], bf16, name=f"rl{j}", tag="rl")
                    for kc in range(2):
                        nc.scalar.activation(
                            rl[:, kc, :w], hT[:, kc, jsl], AF.Relu
                        )
                    dp = hps.tile([1, 512], f32, name=f"dp{j}", tag="dp")
                    for kc in range(2):
                        nc.tensor.matmul(
                            dp[:, :w], wd[:, kc, :], rl[:, kc, :w],
                            start=(kc == 0), stop=(kc == 1),
                        )
                    nc.scalar.copy(d_sb[:, jsl], dp[:, :w])
                # lsm0 = log(sigmoid(d + b_d)), lsm1 = log(sigmoid(-(d + b_d)))
                o0 = hp_pool.tile([1, SHARD], f32, name="o0", bufs=1)
                o1 = hp_pool.tile([1, SHARD], f32, name="o1", bufs=1)
                sg0 = hp_pool.tile([1, SHARD], f32, name="sg0", bufs=1)
                nc.scalar.activation(sg0[:], d_sb[:], AF.Sigmoid, bias=b_d)
                nc.scalar.activation(o0[:], sg0[:], AF.Ln)
                sg1 = hp_pool.tile([1, SHARD], f32, name="sg1", bufs=1)
                nc.scalar.activation(
                    sg1[:], d_sb[:], AF.Sigmoid, scale=-1.0, bias=-b_d
                )
                nc.scalar.activation(o1[:], sg1[:], AF.Ln)
                nc.sync.dma_start(out_t[0:1, :], o0[:])
                nc.sync.dma_start(out_t[1:2, :], o1[:])

    nc.compile()
    return nc


# ------------------------------------------------------------------- driver

def _bf(a):
    return np.asarray(a, dtype=np.float32).astype(ml_dtypes.bfloat16)


def kernel(**inputs):
    x = np.asarray(inputs["x"], dtype=np.float32)
    edge_index = np.asarray(inputs["edge_index"])
    W_reduce = np.asarray(inputs["W_reduce"], dtype=np.float32)
    b_reduce = np.asarray(inputs["b_reduce"], dtype=np.float32)
    W_ggc = np.asarray(inputs["W_ggc"], dtype=np.float32)
    W_ih = np.asarray(inputs["W_ih"], dtype=np.float32)
    W_hh = np.asarray(inputs["W_hh"], dtype=np.float32)
    b_ih = np.asarray(inputs["b_ih"], dtype=np.float32)
    b_hh = np.asarray(inputs["b_hh"], dtype=np.float32)
    W_lin = np.asarray(inputs["W_lin"], dtype=np.float32)
    b_lin = np.asarray(inputs["b_lin"], dtype=np.float32)

    n = x.shape[0]
    caps, percore = _prep_edges(edge_index)
    C0 = int(caps[:, 0].sum())
    C1 = int(caps[:, 1].sum())

    # host-folded weights
    V = np.einsum("sij,kj->sik", W_ggc, W_ih)          # [S, HID, G3]
    V_dev = _bf(V.reshape(STEPS, 2, P, G3).transpose(0, 2, 1, 3))
    whh_dev = _bf(W_hh.T.reshape(2, P, G3).transpose(1, 0, 2))
    wred_pad = np.zeros((HID, HID), np.float32)
    wred_pad[:ANNOT] = W_reduce.T                       # [200->256, 256]
    wred_dev = _bf(wred_pad.reshape(2, P, HID).transpose(1, 0, 2))
    b_rz = (b_ih + b_hh)[:512].reshape(4, P).T.copy()
    b_in = b_ih[512:].reshape(2, P).T.copy()
    b_hn = b_hh[512:].reshape(2, P).T.copy()
    b_red = b_reduce.reshape(2, P).T.copy()
    w_d = (W_lin[0] - W_lin[1]).astype(np.float32)      # [256]
    wd_dev = _bf(w_d.reshape(2, P, 1).transpose(1, 0, 2))
    b_d = float(b_lin[0] - b_lin[1])

    x_pad = np.zeros((N_PAD, ANNOT), np.float32)
    x_pad[:n] = x

    import time as _time

    _t0 = _time.time()
    nc = _build_nc(caps, C0, C1, b_d=b_d)
    print(f"[kernel] build+schedule: {_time.time()-_t0:.1f}s", flush=True)
    _t0 = _time.time()

    in_maps = []
    for c in range(NCORES):
        xs = x_pad[c * SHARD : (c + 1) * SHARD]         # [6400, 200]
        xT = np.zeros((HID, SHARD), np.float32)
        xT[:ANNOT] = xs.T
        m = {
            "xT": _bf(xT.reshape(2, P, SHARD).transpose(1, 0, 2)),
            "w_red": wred_dev,
            "v": V_dev,
            "whh": whh_dev,
            "b_rz": b_rz,
            "b_in": b_in,
            "b_hn": b_hn,
            "b_red": b_red,
            "w_d": wd_dev,
        }
        m.update(percore[c])
        in_maps.append(m)

    res = bass_utils.run_bass_kernel_spmd(
        nc,
        in_maps,
        core_ids=list(range(NCORES)),
        trace=os.environ.get("KERNEL_TRACE") == "1",
    )
    print(f"[kernel] compile+exec: {_time.time()-_t0:.1f}s", flush=True)
    kernel.last_results = res

    full = np.concatenate(
        [res.results[c]["out"] for c in range(NCORES)], axis=1
    )  # [2, N_PAD]
    return full.T[:n].astype(np.float32)

